# revision 38
# baseline (speedup 1.0000x reference)
"""Trainium2 Bass kernel for nn_MultiHeadedSelfAttention_5179730559275.

Reference math (per batch b):
  q = wq @ x + bq ; k = wk @ x + bk ; v = wv @ x + bv        (1x1 conv, C=256 -> O=256)
  per o-channel (o = om*128 + j), with Q_o,K_o,V_o = 64x64 images [H,W]:
    S_o = Q_o @ K_o^T / sqrt(32); P_o = softmax(S_o, axis=-1); ctx_o = P_o @ V_o

Sharding: data-parallel over batch, 2 batches per core on 8 cores.

Per-core pipeline (per batch):
  1. Projection on PE (lhsT = w^T fp16 stationary, rhs = x fp16 moving,
     n=512) -> psum [o', 512]; ACT/DVE evacuate psum + bias -> fp16 SBUF.
     q/k/v are interleaved per pixel-block nt so the psum ring (4 banks)
     never makes the PE wait on an evacuation.  bv is folded into the v
     projection (softmax rows sum to 1, so ctx = P@V0 + bv).
  2. PE transposes (matmul vs identity, fp16 psum) re-lay per-channel
     images with spatial on partitions, pairing channels o and o+128:
       qS/kS: [om*64+w, h, j]   vS: [om*64+g, w|ones, j]
     8 transposes fill one fp16 psum bank [128, 8, 128]; one contiguous
     [128, 1024] DVE copy evacuates it (16-bit 2x mode).  q/k transposes
     lag the projection by one block and are emitted between projection
     matmuls; v transposes run after v16 is complete.
  3. Attention per pair j: quadrant matmuls (K=64 at partition bases 0/64):
       S^T psum [om*64+g, h] ; exp (ACT, bias -2) -> eS fp16
       ctx psum [om*64+h, 0:64]=E^T.T@V, col 64 = Z (ones column)
     DVE: rz = 1/Z, ctx = psum * rz broadcast (bv already in V), DMA out.
     The ctx stage lags the score stage by TWO groups so the exp chain
     (ACT latency) never stalls the PE stream (keeps the p-state ramp).
  Phase order: front(0), attn(0), front(1), attn(1).
"""

import numpy as np

import concourse.bass as bass
import concourse.bacc as bacc
import concourse.tile as tile
from concourse import mybir
from concourse import bass2jax
from concourse.masks import make_identity

NCORES = 8
B, C, H, W = 16, 256, 64, 64
O = 256
PIX = H * W
BL = B // NCORES  # batches per core
SCALE = 1.0 / float(np.sqrt(32.0))
EXP_BIAS = -2.0  # softmax-invariant shift keeping exp() well inside fp16 range

FP32 = mybir.dt.float32
FP16 = mybir.dt.float16


def build_kernel(nc: bass.Bass):
    x_in = nc.declare_dram_parameter("x", [BL, C, PIX], FP16, isOutput=False)
    wT_in = nc.declare_dram_parameter("wT", [3, C, O], FP16, isOutput=False)
    bias_in = nc.declare_dram_parameter("bias", [3, O], FP32, isOutput=False)
    out = nc.declare_dram_parameter("out", [BL, O, PIX], FP16, isOutput=True)

    with tile.TileContext(nc) as tc:
        with (
            tc.tile_pool(name="singles", bufs=1) as singles,
            tc.tile_pool(name="xin", bufs=4) as xpool,
            tc.tile_pool(name="chunks", bufs=2) as chpool,
            tc.tile_pool(name="vfull", bufs=2) as vpool,
            tc.tile_pool(name="tsp", bufs=2) as tpool,
            tc.tile_pool(name="small", bufs=4) as small,
            tc.tile_pool(name="psA", bufs=3, space="PSUM") as psA,
            tc.tile_pool(name="psS", bufs=3, space="PSUM") as psS,
            tc.tile_pool(name="psC", bufs=2, space="PSUM") as psC,
        ):
            # ---- constants loaded once (separate queues for overlap) ----
            w_sb = singles.tile([128, 3, 2, O], FP16)  # [c', proj, cc, o]
            nc.scalar.dma_start(
                out=w_sb,
                in_=wT_in.rearrange("t (cc c) o -> c t cc o", cc=2),
            )
            bias_sb = singles.tile([128, 3, 2], FP32)  # [o', proj, oc]
            nc.scalar.dma_start(
                out=bias_sb,
                in_=bias_in.rearrange("t (oc o) -> o t oc", oc=2),
            )
            # x prefetched in quarter-tiles striped over all DMA queues,
            # cc-interleaved so the first matmul's operands (both c-halves of
            # pixel block 0) arrive first; issued before any gpsimd setup work
            # so the gpsimd DMA ring starts immediately
            xsb = {}
            for b in range(BL):
                for cc in range(2):
                    xsb[(b, cc)] = xpool.tile(
                        [128, PIX], FP16, tag="xsb", name=f"x{b}{cc}"
                    )
            xq = [nc.sync, nc.gpsimd, nc.scalar]
            qi = 0
            for b in range(BL):
                for h in range(4):
                    sl = slice(h * (PIX // 4), (h + 1) * (PIX // 4))
                    for cc in range(2):
                        xq[qi % 3].dma_start(
                            out=xsb[(b, cc)][:, sl],
                            in_=x_in[b, cc * 128 : (cc + 1) * 128, sl],
                        )
                        qi += 1

            expb_sb = singles.tile([128, 1], FP32)
            nc.vector.memset(expb_sb, EXP_BIAS)
            ident = singles.tile([128, 128], FP16)
            make_identity(nc, ident)
            # block-diagonal exp(S^T) tiles for single-instruction ctx matmuls;
            # off-diagonal zero blocks are written once and never touched
            es_bd = []
            for r in range(4):
                eb = singles.tile([128, 8, 128], FP16, name=f"es_bd{r}")
                nc.vector.memset(eb, 0.0)
                es_bd.append(eb)

            tensors = {}
            evac1_ctr = [0]

            def proj_mm(b, proj, oc, nt):
                ps = psA.tile([128, 512], FP32, tag="psA")
                for cc in range(2):
                    nc.tensor.matmul(
                        ps,
                        lhsT=w_sb[:, proj, cc, oc * 128 : (oc + 1) * 128],
                        rhs=xsb[(b, cc)][:, nt * 512 : (nt + 1) * 512],
                        start=(cc == 0),
                        stop=(cc == 1),
                    )
                return ps

            def evac1(dst, ps, proj, oc):
                # psum [o', 512] + bias -> fp16 SBUF (3:1 ACT:DVE round-robin)
                evac1_ctr[0] += 1
                if evac1_ctr[0] % 2 == 0:
                    nc.vector.tensor_scalar_add(
                        out=dst,
                        in0=ps.rearrange("p (h w) -> p h w", w=W),
                        scalar1=bias_sb[:, proj, oc : oc + 1],
                    )
                else:
                    nc.scalar.activation(
                        out=dst,
                        in_=ps.rearrange("p (h w) -> p h w", w=W),
                        func=mybir.ActivationFunctionType.Identity,
                        bias=bias_sb[:, proj, oc : oc + 1],
                        scale=1.0,
                    )

            def qk_transpose(ch, nt, dst):
                # 8 h-rows -> one fp16 psum bank -> one contiguous evac
                pt = psC.tile([128, 8, 128], FP16, tag="psct")
                for i in range(8):
                    # [j, (oc, w)] -> [(oc, w), j] per h row
                    nc.tensor.transpose(
                        pt[:, i, :],
                        ch[:, i, :, :].rearrange("p a b -> p (a b)"),
                        ident,
                    )
                nc.vector.tensor_copy(out=dst[:, nt * 8 : (nt + 1) * 8, :], in_=pt)

            def emit_front(b, interleave=None):
                def pull(n):
                    if interleave is not None:
                        for _ in range(n):
                            next(interleave, None)
                # qS/kS: [om*64+w, h, j]; vS: [om*64+g, w|ones, j]
                qS = tpool.tile([128, H, 128], FP16, tag="qS")
                kS = tpool.tile([128, H, 128], FP16, tag="kS")
                vS = tpool.tile([128, W + 1, 128], FP16, tag="vS")
                nc.vector.memset(vS[:, W, :], 1.0)
                v16 = vpool.tile([128, 2, H, W], FP16, tag="v16")  # [j, oc, g, w]

                pending = {}  # proj -> (chunk, nt) awaiting transpose
                for nt in range(8):
                    chq = chpool.tile([128, 8, 2, W], FP16, tag="ch0")
                    chk = chpool.tile([128, 8, 2, W], FP16, tag="ch1")
                    for proj, ch in ((0, chq), (1, chk)):
                        for oc in range(2):
                            ps = proj_mm(b, proj, oc, nt)
                            evac1(ch[:, :, oc, :], ps, proj, oc)
                    # v projection between q/k matmuls keeps psum ring slack
                    for oc in range(2):
                        ps = proj_mm(b, 2, oc, nt)
                        evac1(v16[:, oc, nt * 8 : (nt + 1) * 8, :], ps, 2, oc)
                    # lagged q/k transposes ride between projection blocks
                    if pending:
                        qk_transpose(pending[0][0], pending[0][1], qS)
                        qk_transpose(pending[1][0], pending[1][1], kS)
                    pending = {0: (chq, nt), 1: (chk, nt)}
                    # low-power attention groups of the previous batch spread
                    # between power-heavy projection blocks (power smoothing)
                    pull(2)
                qk_transpose(pending[0][0], pending[0][1], qS)
                qk_transpose(pending[1][0], pending[1][1], kS)

                for vg in range(8):
                    pt = psC.tile([128, 8, 128], FP16, tag="psct")
                    for i in range(8):
                        w = vg * 8 + i
                        # [j, (oc, g)] -> [(oc, g), j] per w column
                        nc.tensor.transpose(
                            pt[:, i, :],
                            v16[:, :, :, w].rearrange("p a b -> p (a b)"),
                            ident,
                        )
                    nc.vector.tensor_copy(
                        out=vS[:, vg * 8 : (vg + 1) * 8, :], in_=pt
                    )
                tensors[b] = (qS, kS, vS)

            def attn_steps(b):
                qS, kS, vS = tensors[b]
                JG = 8
                pending = []  # [(jg, eS8), ...] ctx stages, lagged by 2

                def emit_ctx(jg, eS8):
                    oc8 = small.tile([128, JG, W], FP16, tag="oc8")
                    for sg in range(2):
                        cp4f = psC.tile([128, 512], FP32, tag="psct")
                        cp4 = cp4f[:, 0 : 4 * (W + 1)].rearrange(
                            "p (i c) -> p i c", c=W + 1
                        )
                        for i in range(4):
                            j = jg * JG + sg * 4 + i
                            # block-diagonal eS covers both channel quadrants
                            nc.tensor.matmul(
                                cp4[:, i, :],
                                lhsT=eS8[:, sg * 4 + i, :],
                                rhs=vS[:, :, j],
                                start=True,
                                stop=True,
                            )
                        rz4 = small.tile([128, 4], FP32, tag="rz4")
                        nc.vector.reciprocal(out=rz4, in_=cp4[:, :, W])
                        nc.vector.tensor_tensor(
                            oc8[:, sg * 4 : (sg + 1) * 4, :],
                            cp4[:, :, 0:W],
                            rz4[:, :, None].to_broadcast([128, 4, W]),
                            mybir.AluOpType.mult,
                        )
                    # split output DMAs across the sync and (idle) gpsimd
                    # queues so each group's writeback issues in parallel
                    for om, q in ((0, nc.sync), (1, nc.gpsimd)):
                        j0 = jg * JG
                        q.dma_start(
                            out=out[
                                b, om * 128 + j0 : om * 128 + j0 + JG, :
                            ].rearrange("j (h w) -> h j w", w=W),
                            in_=oc8[om * 64 : om * 64 + 64, :, :],
                        )

                for jg in range(16):
                    sp8f = psS.tile([128, 512], FP32, tag="psS")
                    sp8 = sp8f.rearrange("p (i h) -> p i h", h=H)
                    for i in range(JG):
                        j = jg * JG + i
                        for om in range(2):
                            pr = slice(om * 64, om * 64 + 64)
                            nc.tensor.matmul(
                                sp8[pr, i, :],
                                lhsT=kS[pr, :, j],
                                rhs=qS[pr, :, j],
                                start=True,
                                stop=True,
                            )
                    eS8 = es_bd[(b * 16 + jg) % 4]
                    for om in range(2):
                        pr = slice(om * 64, om * 64 + 64)
                        nc.scalar.activation(
                            out=eS8[pr, :, om * 64 : om * 64 + 64],
                            in_=sp8[pr, :, :],
                            func=mybir.ActivationFunctionType.Exp,
                            bias=expb_sb[pr, :],
                            scale=1.0,
                        )
                    pending.append((jg, eS8))
                    if len(pending) > 2:
                        emit_ctx(*pending.pop(0))
                    yield
                for st in pending:
                    emit_ctx(*st)

            emit_front(0)
            g0 = attn_steps(0)
            if BL > 1:
                emit_front(1, interleave=g0)
            for _ in g0:
                pass
            if BL > 1:
                for _ in attn_steps(1):
                    pass
    return nc


_NC_CACHE = {}


def get_nc():
    if "nc" not in _NC_CACHE:
        nc = bacc.Bacc(None, target_bir_lowering=False)
        build_kernel(nc)
        nc.finalize()
        _NC_CACHE["nc"] = nc
    return _NC_CACHE["nc"]


def prep_in_maps(x, wq, bq, wk, bk, wv, bv):
    wT = np.stack(
        [
            np.ascontiguousarray((wq * SCALE).T),
            np.ascontiguousarray(wk.T),
            np.ascontiguousarray(wv.T),
        ]
    ).astype(np.float16)
    # bv rides in the v projection: softmax rows sum to 1, so ctx = P@V0 + bv
    biases = np.stack([bq * SCALE, bk, bv]).astype(np.float32)
    xs = np.ascontiguousarray(x.reshape(NCORES, BL, C, PIX)).astype(np.float16)
    return [{"x": xs[i], "wT": wT, "bias": biases} for i in range(NCORES)]


def kernel(x, wq, bq, wk, bk, wv, bv):
    nc = get_nc()
    in_maps = prep_in_maps(x, wq, bq, wk, bk, wv, bv)
    results = bass2jax.run_bass_via_pjrt(nc, in_maps, n_cores=NCORES)
    outs = [np.asarray(r["out"]).reshape(BL, O, H, W) for r in results]
    return np.concatenate(outs, axis=0).astype(np.float32)


# revision 43
# speedup vs baseline: 1.2398x; 1.2398x over previous
"""Trainium2 Bass kernel for nn_MultiHeadedSelfAttention_5179730559275.

Reference math (per batch b):
  q = wq @ x + bq ; k = wk @ x + bk ; v = wv @ x + bv        (1x1 conv, C=256 -> O=256)
  per o-channel (o = om*128 + j), with Q_o,K_o,V_o = 64x64 images [H,W]:
    S_o = Q_o @ K_o^T / sqrt(32); P_o = softmax(S_o, axis=-1); ctx_o = P_o @ V_o

Sharding: data-parallel over batch, 2 batches per core on 8 cores.

Per-core pipeline (per batch):
  1. Projection on PE (lhsT = w^T fp16 stationary, rhs = x fp16 moving,
     n=512) -> psum [o', 512]; ACT/DVE evacuate psum + bias -> fp16 SBUF.
     q/k/v are interleaved per pixel-block nt so the psum ring (4 banks)
     never makes the PE wait on an evacuation.  bv is folded into the v
     projection (softmax rows sum to 1, so ctx = P@V0 + bv).
  2. PE transposes (matmul vs identity, fp16 psum) re-lay per-channel
     images with spatial on partitions, pairing channels o and o+128:
       qS/kS: [om*64+w, h, j]   vS: [om*64+g, w|ones, j]
     8 transposes fill one fp16 psum bank [128, 8, 128]; one contiguous
     [128, 1024] DVE copy evacuates it (16-bit 2x mode).  q/k transposes
     lag the projection by one block and are emitted between projection
     matmuls; v transposes run after v16 is complete.
  3. Attention per pair j: quadrant matmuls (K=64 at partition bases 0/64):
       S^T psum [om*64+g, h] ; exp (ACT, bias -2) -> eS fp16
       ctx psum [om*64+h, 0:64]=E^T.T@V, col 64 = Z (ones column)
     DVE: rz = 1/Z, ctx = psum * rz broadcast (bv already in V), DMA out.
     The ctx stage lags the score stage by TWO groups so the exp chain
     (ACT latency) never stalls the PE stream (keeps the p-state ramp).
  Phase order: front(0), attn(0), front(1), attn(1).
"""

import numpy as np

import concourse.bass as bass
import concourse.bacc as bacc
import concourse.tile as tile
from concourse import mybir
from concourse import bass2jax
from concourse.masks import make_identity

NCORES = 8
B, C, H, W = 16, 256, 64, 64
O = 256
PIX = H * W
BL = B // NCORES  # batches per core
SCALE = 1.0 / float(np.sqrt(32.0))
EXP_BIAS = -2.0  # softmax-invariant shift keeping exp() well inside fp16 range

FP32 = mybir.dt.float32
FP16 = mybir.dt.float16


def build_kernel(nc: bass.Bass):
    x_in = nc.declare_dram_parameter("x", [BL, C, PIX], FP16, isOutput=False)
    wT_in = nc.declare_dram_parameter("wT", [3, C, O], FP16, isOutput=False)
    bias_in = nc.declare_dram_parameter("bias", [3, O], FP32, isOutput=False)
    out = nc.declare_dram_parameter("out", [BL, O, PIX], FP16, isOutput=True)

    with tile.TileContext(nc) as tc:
        with (
            tc.tile_pool(name="singles", bufs=1) as singles,
            tc.tile_pool(name="xin", bufs=4) as xpool,
            tc.tile_pool(name="chunks", bufs=2) as chpool,
            tc.tile_pool(name="vfull", bufs=2) as vpool,
            tc.tile_pool(name="tsp", bufs=2) as tpool,
            tc.tile_pool(name="small", bufs=4) as small,
            tc.tile_pool(name="psA", bufs=3, space="PSUM") as psA,
            tc.tile_pool(name="psS", bufs=3, space="PSUM") as psS,
            tc.tile_pool(name="psC", bufs=2, space="PSUM") as psC,
        ):
            # ---- constants loaded once (separate queues for overlap) ----
            w_sb = singles.tile([128, 3, 2, O], FP16)  # [c', proj, cc, o]
            nc.scalar.dma_start(
                out=w_sb,
                in_=wT_in.rearrange("t (cc c) o -> c t cc o", cc=2),
            )
            bias_sb = singles.tile([128, 3, 2], FP32)  # [o', proj, oc]
            nc.scalar.dma_start(
                out=bias_sb,
                in_=bias_in.rearrange("t (oc o) -> o t oc", oc=2),
            )
            # x prefetched in quarter-tiles striped over all DMA queues,
            # cc-interleaved so the first matmul's operands (both c-halves of
            # pixel block 0) arrive first; issued before any gpsimd setup work
            # so the gpsimd DMA ring starts immediately
            xsb = {}
            for b in range(BL):
                for cc in range(2):
                    xsb[(b, cc)] = xpool.tile(
                        [128, PIX], FP16, tag="xsb", name=f"x{b}{cc}"
                    )
            xq = [nc.sync, nc.gpsimd, nc.scalar]
            qi = 0
            for b in range(BL):
                for h in range(4):
                    sl = slice(h * (PIX // 4), (h + 1) * (PIX // 4))
                    for cc in range(2):
                        xq[qi % 3].dma_start(
                            out=xsb[(b, cc)][:, sl],
                            in_=x_in[b, cc * 128 : (cc + 1) * 128, sl],
                        )
                        qi += 1

            expb_sb = singles.tile([128, 1], FP32)
            nc.vector.memset(expb_sb, EXP_BIAS)
            ident = singles.tile([128, 128], FP16)
            make_identity(nc, ident)
            # block-diagonal exp(S^T) tiles for single-instruction ctx matmuls;
            # off-diagonal zero blocks are written once and never touched
            es_bd = []
            for r in range(4):
                eb = singles.tile([128, 8, 128], FP16, name=f"es_bd{r}")
                nc.vector.memset(eb, 0.0)
                es_bd.append(eb)

            tensors = {}
            evac1_ctr = [0]

            def proj_mm(b, proj, oc, nt):
                ps = psA.tile([128, 512], FP32, tag="psA")
                for cc in range(2):
                    nc.tensor.matmul(
                        ps,
                        lhsT=w_sb[:, proj, cc, oc * 128 : (oc + 1) * 128],
                        rhs=xsb[(b, cc)][:, nt * 512 : (nt + 1) * 512],
                        start=(cc == 0),
                        stop=(cc == 1),
                    )
                return ps

            def evac1(dst, ps, proj, oc):
                # psum [o', 512] + bias -> fp16 SBUF (3:1 ACT:DVE round-robin)
                evac1_ctr[0] += 1
                if evac1_ctr[0] % 4 == 0:
                    nc.vector.tensor_scalar_add(
                        out=dst,
                        in0=ps.rearrange("p (h w) -> p h w", w=W),
                        scalar1=bias_sb[:, proj, oc : oc + 1],
                    )
                else:
                    nc.scalar.activation(
                        out=dst,
                        in_=ps.rearrange("p (h w) -> p h w", w=W),
                        func=mybir.ActivationFunctionType.Identity,
                        bias=bias_sb[:, proj, oc : oc + 1],
                        scale=1.0,
                    )

            def qk_transpose(ch, nt, dst):
                # 8 h-rows -> one fp16 psum bank -> one contiguous evac
                pt = psC.tile([128, 8, 128], FP16, tag="psct")
                for i in range(8):
                    # [j, (oc, w)] -> [(oc, w), j] per h row
                    nc.tensor.transpose(
                        pt[:, i, :],
                        ch[:, i, :, :].rearrange("p a b -> p (a b)"),
                        ident,
                    )
                nc.vector.tensor_copy(out=dst[:, nt * 8 : (nt + 1) * 8, :], in_=pt)

            def emit_front(b):
                # qS/kS: [om*64+w, h, j]; vS: [om*64+g, w|ones, j]
                qS = tpool.tile([128, H, 128], FP16, tag="qS")
                kS = tpool.tile([128, H, 128], FP16, tag="kS")
                vS = tpool.tile([128, W + 1, 128], FP16, tag="vS")
                nc.vector.memset(vS[:, W, :], 1.0)
                v16 = vpool.tile([128, 2, H, W], FP16, tag="v16")  # [j, oc, g, w]

                pending = {}  # proj -> (chunk, nt) awaiting transpose
                for nt in range(8):
                    chq = chpool.tile([128, 8, 2, W], FP16, tag="ch0")
                    chk = chpool.tile([128, 8, 2, W], FP16, tag="ch1")
                    for proj, ch in ((0, chq), (1, chk)):
                        for oc in range(2):
                            ps = proj_mm(b, proj, oc, nt)
                            evac1(ch[:, :, oc, :], ps, proj, oc)
                    # v projection between q/k matmuls keeps psum ring slack
                    for oc in range(2):
                        ps = proj_mm(b, 2, oc, nt)
                        evac1(v16[:, oc, nt * 8 : (nt + 1) * 8, :], ps, 2, oc)
                    # lagged q/k transposes ride between projection blocks
                    if pending:
                        qk_transpose(pending[0][0], pending[0][1], qS)
                        qk_transpose(pending[1][0], pending[1][1], kS)
                    pending = {0: (chq, nt), 1: (chk, nt)}
                qk_transpose(pending[0][0], pending[0][1], qS)
                qk_transpose(pending[1][0], pending[1][1], kS)

                for vg in range(8):
                    pt = psC.tile([128, 8, 128], FP16, tag="psct")
                    for i in range(8):
                        w = vg * 8 + i
                        # [j, (oc, g)] -> [(oc, g), j] per w column
                        nc.tensor.transpose(
                            pt[:, i, :],
                            v16[:, :, :, w].rearrange("p a b -> p (a b)"),
                            ident,
                        )
                    nc.vector.tensor_copy(
                        out=vS[:, vg * 8 : (vg + 1) * 8, :], in_=pt
                    )
                tensors[b] = (qS, kS, vS)

            def emit_attn(b):
                qS, kS, vS = tensors[b]
                JG = 8
                pending = []  # [(jg, eS8), ...] ctx stages, lagged by 2

                def emit_ctx(jg, eS8):
                    oc8 = small.tile([128, JG, W], FP16, tag="oc8")
                    for sg in range(2):
                        cp4f = psC.tile([128, 512], FP32, tag="psct")
                        cp4 = cp4f[:, 0 : 4 * (W + 1)].rearrange(
                            "p (i c) -> p i c", c=W + 1
                        )
                        for i in range(4):
                            j = jg * JG + sg * 4 + i
                            # block-diagonal eS covers both channel quadrants
                            nc.tensor.matmul(
                                cp4[:, i, :],
                                lhsT=eS8[:, sg * 4 + i, :],
                                rhs=vS[:, :, j],
                                start=True,
                                stop=True,
                            )
                        rz4 = small.tile([128, 4], FP32, tag="rz4")
                        nc.vector.reciprocal(out=rz4, in_=cp4[:, :, W])
                        nc.vector.tensor_tensor(
                            oc8[:, sg * 4 : (sg + 1) * 4, :],
                            cp4[:, :, 0:W],
                            rz4[:, :, None].to_broadcast([128, 4, W]),
                            mybir.AluOpType.mult,
                        )
                    # split output DMAs across the sync and (idle) gpsimd
                    # queues so each group's writeback issues in parallel
                    for om, q in ((0, nc.sync), (1, nc.gpsimd)):
                        j0 = jg * JG
                        q.dma_start(
                            out=out[
                                b, om * 128 + j0 : om * 128 + j0 + JG, :
                            ].rearrange("j (h w) -> h j w", w=W),
                            in_=oc8[om * 64 : om * 64 + 64, :, :],
                        )

                for jg in range(16):
                    sp8f = psS.tile([128, 512], FP32, tag="psS")
                    sp8 = sp8f.rearrange("p (i h) -> p i h", h=H)
                    for i in range(JG):
                        j = jg * JG + i
                        for om in range(2):
                            pr = slice(om * 64, om * 64 + 64)
                            nc.tensor.matmul(
                                sp8[pr, i, :],
                                lhsT=kS[pr, :, j],
                                rhs=qS[pr, :, j],
                                start=True,
                                stop=True,
                            )
                    eS8 = es_bd[(b * 16 + jg) % 4]
                    for om in range(2):
                        pr = slice(om * 64, om * 64 + 64)
                        nc.scalar.activation(
                            out=eS8[pr, :, om * 64 : om * 64 + 64],
                            in_=sp8[pr, :, :],
                            func=mybir.ActivationFunctionType.Exp,
                            bias=expb_sb[pr, :],
                            scale=1.0,
                        )
                    pending.append((jg, eS8))
                    if len(pending) > 2:
                        emit_ctx(*pending.pop(0))
                for st in pending:
                    emit_ctx(*st)

            emit_front(0)
            emit_attn(0)
            if BL > 1:
                emit_front(1)
                emit_attn(1)
    return nc


_NC_CACHE = {}


def get_nc():
    if "nc" not in _NC_CACHE:
        nc = bacc.Bacc(None, target_bir_lowering=False)
        build_kernel(nc)
        nc.finalize()
        _NC_CACHE["nc"] = nc
    return _NC_CACHE["nc"]


def prep_in_maps(x, wq, bq, wk, bk, wv, bv):
    wT = np.stack(
        [
            np.ascontiguousarray((wq * SCALE).T),
            np.ascontiguousarray(wk.T),
            np.ascontiguousarray(wv.T),
        ]
    ).astype(np.float16)
    # bv rides in the v projection: softmax rows sum to 1, so ctx = P@V0 + bv
    biases = np.stack([bq * SCALE, bk, bv]).astype(np.float32)
    xs = np.ascontiguousarray(x.reshape(NCORES, BL, C, PIX)).astype(np.float16)
    return [{"x": xs[i], "wT": wT, "bias": biases} for i in range(NCORES)]


def kernel(x, wq, bq, wk, bk, wv, bv):
    nc = get_nc()
    in_maps = prep_in_maps(x, wq, bq, wk, bk, wv, bv)
    results = bass2jax.run_bass_via_pjrt(nc, in_maps, n_cores=NCORES)
    outs = [np.asarray(r["out"]).reshape(BL, O, H, W) for r in results]
    return np.concatenate(outs, axis=0).astype(np.float32)
